# revision 1
# baseline (speedup 1.0000x reference)
"""Trainium2 Bass kernel for the quantized ResNet Bottleneck block.

Sharding: data-parallel over batch across 8 NeuronCores (8 images/core),
no collectives. Inside each core:

  conv1 (1x1, 1024->256): out = sum_k I1[co,ci] * (xh + xl)[ci,n]
      where I1 = round(w1/s1) are integers in [-127,127] held in bf16
      (exactly representable) and xh/xl is an exact bf16 hi/lo encoding
      of the fp32 input (same total bytes; pure re-encoding done on the
      host during input marshalling). Products are exact in the
      fp32-accumulating PE, so conv1 is fp32-grade.
  BN+PACT fold to per-channel affine in "u units" (u = y*255/alpha):
      u = A*S + B; r = round(clip(u,0,255)) via the fp32 magic-number
      trick (+1.5*2^23, clip to [C,C+255], -1.5*2^23) = exact RNE.
      Activations stay integers r in bf16 (exact).
  conv2 (3x3, pad 1): 9 shifted-window matmuls over a zero-padded
      [ci, b, 16, 16] buffer, integer weights I2.
  conv3 (1x1, 256->1024): integer weights I3; epilogue adds the fp32
      residual x and applies PACT3.

All model math (weight quant scales+rounding, BN folding, convs, PACT)
runs on device. Host only slices/transposes/re-encodes inputs and reads
the a1/a2 scalars (asserted constant) to bake fold constants.
"""
import sys
from contextlib import ExitStack
sys.path.insert(0, '/opt/trn_rl_repo')

import numpy as np
import ml_dtypes
import concourse.bass as bass
import concourse.mybir as mybir
from concourse import bacc
from concourse.tile import TileContext
from concourse.bass_utils import run_bass_kernel_spmd
from concourse.masks import make_identity

F32 = mybir.dt.float32
BF16 = mybir.dt.bfloat16
AF = mybir.ActivationFunctionType
ALU = mybir.AluOpType
AX = mybir.AxisListType

MAGIC = float(np.float32(12582912.0))  # 1.5 * 2**23
EPS = 1e-5

B = 8            # images per core
HW = 196         # 14*14
NT = 4           # n-tiles (2 images each)
NS = 392         # free size per n-tile
ROW = NT * NS    # 1568
WID = 256
CIN = 1024
COUT = 1024
KP1 = CIN // 128
MP1 = WID // 128
MP3 = COUT // 128
GRP = 2             # image groups per core


def _quant_layer(nc, work, w_tiles, s_st, rs_st, ipre_tiles, free):
    """Per-output-channel symmetric int8 fake-quant in co-on-partition
    layout: s = max(amax/127, 1e-8); I = round(w * (1/s)) as bf16 ints.
    reduce+recip+mul on DVE, round->bf16 on GPSIMD."""
    for p, (wt, ip) in enumerate(zip(w_tiles, ipre_tiles)):
        am = work.tile([128, 1], F32, tag='qam', name=f'qam_{p}_{free}')
        nc.vector.tensor_reduce(am, wt, axis=AX.X, op=ALU.max,
                                apply_absolute_value=True)
        nc.vector.tensor_scalar(s_st[:, p:p + 1], am, 1.0 / 127.0, 1e-8,
                                op0=ALU.mult, op1=ALU.max)
        nc.vector.reciprocal(rs_st[:, p:p + 1], s_st[:, p:p + 1])
        qt = work.tile([128, free], F32, tag='qtmp', name=f'qt_{p}_{free}',
                       padded_shape=[128, WID * 9], bufs=1)
        nc.vector.tensor_scalar(qt, wt, rs_st[:, p:p + 1], MAGIC,
                                op0=ALU.mult, op1=ALU.add)
        nc.gpsimd.tensor_scalar(ip, qt, MAGIC, None, op0=ALU.subtract)


def _bn_fold(nc, data, g, b_, m, v, a, s_st, nmul, name):
    """Per-channel fold constants, stacked [128, P].
    A = s_w*inv*(255/a)*nmul; Bc = (b - m*inv)*(255/a); K = 255/a;
    S = a/255; inv = g/sqrt(v+EPS) with one Newton step on sqrt."""
    P = g.shape[1]

    def t(nm):
        return data.tile([128, P], F32, name=f'{nm}_{name}')

    eps_col = data.tile([128, 1], F32, name=f'eps_{name}')
    nc.gpsimd.memset(eps_col, EPS)
    ve = t('ve')
    nc.gpsimd.tensor_scalar(ve, v, EPS, None, op0=ALU.add)
    sq0, rq, q, sq, rsq = t('sq0'), t('rq'), t('q'), t('sq'), t('rsq')
    nc.scalar.activation(sq0, v, AF.Sqrt, bias=eps_col, scale=1.0)
    # Newton: sq = 0.5*(sq0 + ve/sq0) — kills the ACT spline error
    nc.vector.reciprocal(rq, sq0)
    nc.gpsimd.tensor_mul(q, ve, rq)
    nc.gpsimd.tensor_add(sq, sq0, q)
    nc.gpsimd.tensor_scalar(sq, sq, 0.5, None, op0=ALU.mult)
    nc.vector.reciprocal(rsq, sq)
    inv, mb, beta, ra, k255, A, Bc, Sc = (t('inv'), t('mb'), t('beta'),
                                          t('ra'), t('k255'), t('A'),
                                          t('Bc'), t('Sc'))
    nc.gpsimd.tensor_mul(inv, g, rsq)
    nc.gpsimd.tensor_mul(mb, m, inv)
    nc.gpsimd.tensor_sub(beta, b_, mb)
    nc.vector.reciprocal(ra, a)
    nc.gpsimd.tensor_scalar(k255, ra, 255.0, None, op0=ALU.mult)
    nc.gpsimd.tensor_mul(A, inv, k255)
    nc.gpsimd.tensor_mul(A, A, s_st)
    if nmul != 1.0:
        nc.gpsimd.tensor_scalar(A, A, nmul, None, op0=ALU.mult)
    nc.gpsimd.tensor_mul(Bc, beta, k255)
    nc.gpsimd.tensor_scalar(Sc, a, 1.0 / 255.0, None, op0=ALU.mult)
    return A, Bc, k255, Sc


def build_nc(a1c, a2c, a3c):
    nc = bacc.Bacc(trn_type='TRN2')

    # host-relaid-out inputs: x [CIN,B,HW] fp32 (residual), xh/xl bf16
    # hi/lo encoding of the same values (conv1 input), all contiguous.
    x_d = nc.dram_tensor('x', [CIN, B, HW], F32, kind='ExternalInput')
    xhl_d = nc.dram_tensor('xhl', [CIN, 2, B, HW], BF16,
                           kind='ExternalInput')
    w1_d = nc.dram_tensor('w1', [WID, CIN], F32, kind='ExternalInput')
    w2_d = nc.dram_tensor('w2', [WID, WID * 9], F32, kind='ExternalInput')
    w3_d = nc.dram_tensor('w3', [COUT, WID], F32, kind='ExternalInput')
    pr = {}
    for l, c in (('1', WID), ('2', WID), ('3', COUT)):
        pr['p' + l] = nc.dram_tensor('p' + l, [128, 5 * (c // 128)], F32,
                                     kind='ExternalInput')
    out_d = nc.dram_tensor('out', [COUT, B, HW], F32, kind='ExternalOutput')

    nc._phase_marks = []

    def mark(nm):
        nc._phase_marks.append((nm, len(nc.inst_map)))

    with TileContext(nc, pool_alloc_mode='queue') as tc:
        xstack = ExitStack()
        with tc.tile_pool(name='data', bufs=1) as data, \
             tc.tile_pool(name='work', bufs=2) as work, \
             tc.tile_pool(name='ps', bufs=6, space='PSUM') as ps, \
             tc.tile_pool(name='pst', bufs=2, space='PSUM') as pst:
            xsplit = xstack.enter_context(tc.tile_pool(name='xsplit', bufs=1))

            ident = data.tile([128, 128], BF16, name='ident')
            make_identity(nc, ident)
            pad1 = [data.tile([128, B, 16, 16], BF16, name=f'pad1_{p}')
                    for p in range(MP1)]

            # ---- DMA schedule: W1, params, all of x, W2, W3 ----
            def load_w(wd, p, free):
                wt = work.tile([128, free], F32,
                               name=f'wraw_{wd.name}_{p}', bufs=1)
                nc.sync.dma_start(wt, wd[p * 128:(p + 1) * 128, :])
                return wt

            W1 = [load_w(w1_d, p, CIN) for p in range(MP1)]
            st = {}
            for l, P in (('1', MP1), ('2', MP1), ('3', MP3)):
                tl = data.tile([128, 5 * P], F32, name=f'pstk{l}')
                nc.sync.dma_start(tl, pr['p' + l][:, :])
                for i, nm in enumerate(('g', 'b', 'm', 'v', 'a')):
                    st[nm + l] = tl[:, i * P:(i + 1) * P]
            xhl = [xsplit.tile([128, 2, B, HW], BF16, name=f'xhl_{k}')
                   for k in range(KP1)]
            xh = [t[:, 0] for t in xhl]
            xl = [t[:, 1] for t in xhl]
            W2 = []
            W3 = []
            for k in range(KP1):
                nc.sync.dma_start(xhl[k],
                                  xhl_d[k * 128:(k + 1) * 128, :, :, :])
                if k == 0:
                    W2 = [load_w(w2_d, p, WID * 9) for p in range(MP1)]
                elif k == 1:
                    W3 = [load_w(w3_d, p, WID) for p in range(MP3)]

            # ---- layer-1 quant + fold + transpose ----
            s1 = data.tile([128, MP1], F32, name='s1st')
            rs1 = data.tile([128, MP1], F32, name='rs1st')
            I1p = [work.tile([128, CIN], BF16, tag='ipre', name=f'I1p_{p}',
                             padded_shape=[128, WID * 9])
                   for p in range(MP1)]
            _quant_layer(nc, work, W1, s1, rs1, I1p, CIN)
            A1, B1, _, _ = _bn_fold(nc, data, st['g1'], st['b1'], st['m1'],
                                    st['v1'], st['a1'], s1, 1.0, 'l1')
            I1Tb = [data.tile([128, 2 * WID], BF16, name=f'I1Tb_{j}')
                    for j in range(KP1 // 2)]
            I1T = []
            for j in range(KP1 // 2):
                I1T += [I1Tb[j][:, 0:WID], I1Tb[j][:, WID:2 * WID]]
            for j in range(KP1 // 2):
                pt = pst.tile([128, 2 * WID], BF16, tag='pst',
                              name=f'ptr1_{j}')
                for h in range(2):
                    k = 2 * j + h
                    for mp in range(MP1):
                        nc.tensor.transpose(
                            pt[:, h * WID + mp * 128:
                               h * WID + (mp + 1) * 128],
                            I1p[mp][:, k * 128:(k + 1) * 128], ident)
                nc.scalar.copy(I1Tb[j], pt)

            def ep12(psb, A, Bc, mp, out_bf16_ap):
                """BN+PACT epilogue: per-bank ACT relu -> row DVE clip
                -> row GP unmagic to bf16 ints."""
                t_row = work.tile([128, ROW], F32, tag='rowA',
                                  name=f't_{mp}', bufs=3)
                for n in range(NT):
                    nc.scalar.activation(t_row[:, n * NS:(n + 1) * NS],
                                         psb[n], AF.Relu,
                                         bias=Bc[:, mp:mp + 1],
                                         scale=A[:, mp:mp + 1])
                d_row = work.tile([128, ROW], F32, tag='rowB',
                                  name=f'd_{mp}', bufs=3)
                nc.vector.tensor_scalar(d_row, t_row, MAGIC, MAGIC + 255.0,
                                        op0=ALU.add, op1=ALU.min)
                nc.gpsimd.tensor_scalar(out_bf16_ap, d_row if
                                        len(out_bf16_ap.shape) == 2 else
                                        d_row.rearrange(
                                            'p (b y x) -> p b y x',
                                            b=B, y=14),
                                        MAGIC, None, op0=ALU.subtract)

            mark('prep2')
            # ---- layer-2 quant + fold + per-block transpose ----
            s2 = data.tile([128, MP1], F32, name='s2st')
            rs2 = data.tile([128, MP1], F32, name='rs2st')
            I2p = [work.tile([128, WID * 9], BF16, tag='ipre',
                             name=f'I2p_{p}')
                   for p in range(MP1)]
            _quant_layer(nc, work, W2, s2, rs2, I2p, WID * 9)
            A2, B2, _, _ = _bn_fold(nc, data, st['g2'], st['b2'], st['m2'],
                                    st['v2'], st['a2'], s2, a1c / 255.0, 'l2')
            mark('prep3')
            # ---- layer-3 quant + fold + transpose ----
            s3 = data.tile([128, MP3], F32, name='s3st')
            rs3 = data.tile([128, MP3], F32, name='rs3st')
            I3p = [work.tile([128, WID], BF16, tag='i3pre', name=f'I3p_{p}',
                             bufs=8)
                   for p in range(MP3)]
            _quant_layer(nc, work, W3, s3, rs3, I3p, WID)
            A3, B3, K3, S3 = _bn_fold(nc, data, st['g3'], st['b3'], st['m3'],
                                      st['v3'], st['a3'], s3, a2c / 255.0,
                                      'l3')
            mark('conv1')
            # ---- group-pipelined conv1/conv2/conv3 over 2 image groups ----
            NG = NT // GRP        # n-tiles per group
            GROW = NG * NS        # row elems per group (784)
            k3c = 255.0 / a3c
            r2 = [data.tile([128, ROW], BF16, name=f'r2_{p}')
                  for p in range(MP1)]

            def ep12g(psb, A, Bc, mp, g, pad_out, r2_out):
                t_row = work.tile([128, GROW], F32, tag='rowA',
                                  name=f't_{mp}_{g}', bufs=4)
                for i, n in enumerate(range(g * NG, (g + 1) * NG)):
                    nc.scalar.activation(t_row[:, i * NS:(i + 1) * NS],
                                         psb[i], AF.Relu,
                                         bias=Bc[:, mp:mp + 1],
                                         scale=A[:, mp:mp + 1])
                d_row = work.tile([128, GROW], F32, tag='rowB',
                                  name=f'd_{mp}_{g}', bufs=4)
                nc.vector.tensor_scalar(d_row, t_row, MAGIC, MAGIC + 255.0,
                                        op0=ALU.add, op1=ALU.min)
                if pad_out is not None:
                    nc.gpsimd.tensor_scalar(
                        pad_out, d_row.rearrange('p (b y x) -> p b y x',
                                                 b=2 * NG, y=14),
                        MAGIC, None, op0=ALU.subtract)
                else:
                    nc.gpsimd.tensor_scalar(r2_out, d_row, MAGIC, None,
                                            op0=ALU.subtract)

            def conv1g(g):
                for mp in range(MP1):
                    psb = [ps.tile([128, NS], F32, tag='ps',
                                   name=f'ps1_{mp}_{g}_{i}')
                           for i in range(NG)]
                    for k in range(KP1):
                        lhs = I1T[k][:, mp * 128:(mp + 1) * 128]
                        for i, n in enumerate(range(g * NG, (g + 1) * NG)):
                            nc.tensor.matmul(psb[i], lhs,
                                             xh[k][:, 2 * n:2 * n + 2, :],
                                             start=(k == 0), stop=False)
                            nc.tensor.matmul(psb[i], lhs,
                                             xl[k][:, 2 * n:2 * n + 2, :],
                                             start=False,
                                             stop=(k == KP1 - 1))
                    ep12g(psb, A1, B1, mp, g,
                          pad1[mp][:, 2 * NG * g:2 * NG * (g + 1), 1:15, 1:15], None)

            def conv2g(g, mp_lo=0, mp_hi=MP1):
                for mp in range(mp_lo, mp_hi):
                    psb = [ps.tile([128, NS], F32, tag='ps',
                                   name=f'ps2_{mp}_{g}_{i}')
                           for i in range(NG)]
                    for tap in range(9):
                        dy, dx = tap // 3, tap % 3
                        for ci in range(MP1):
                            lhs = I2T[(tap, ci, mp)]
                            for i, n in enumerate(
                                    range(g * NG, (g + 1) * NG)):
                                nc.tensor.matmul(
                                    psb[i], lhs,
                                    pad1[ci][:, 2 * n:2 * n + 2,
                                             dy:dy + 14, dx:dx + 14],
                                    start=(tap == 0 and ci == 0),
                                    stop=(tap == 8 and ci == MP1 - 1))
                    ep12g(psb, A2, B2, mp, g, None,
                          r2[mp][:, g * GROW:(g + 1) * GROW])

            def conv3g(g, mp_lo=0, mp_hi=MP3):
                PF = 4

                def load_xr(mp):
                    xr = work.tile([128, 2 * NG, HW], F32, tag='xio',
                                   name=f'xr_{mp}_{g}', bufs=5)
                    nc.sync.dma_start(
                        xr, x_d[mp * 128:(mp + 1) * 128,
                                2 * NG * g:2 * NG * (g + 1), :])
                    return xr

                xrs = {mp: load_xr(mp)
                       for mp in range(mp_lo, min(mp_hi, mp_lo + PF))}
                for mp in range(mp_lo, mp_hi):
                    if mp + PF < mp_hi:
                        xrs[mp + PF] = load_xr(mp + PF)
                    xr = xrs.pop(mp)
                    psb = [ps.tile([128, NS], F32, tag='ps',
                                   name=f'ps3_{mp}_{g}_{i}')
                           for i in range(NG)]
                    for ci in range(MP1):
                        lhs = I3T[ci][:, mp * 128:(mp + 1) * 128]
                        for i, n in enumerate(range(g * NG, (g + 1) * NG)):
                            nc.tensor.matmul(
                                psb[i], lhs,
                                r2[ci][:, n * NS:(n + 1) * NS],
                                start=(ci == 0), stop=(ci == MP1 - 1))
                    # last-group last tiles: bank-granular chains to cut
                    # the end-of-kernel epilogue tail
                    fine = (g == GRP - 1 and mp >= MP3 - 2 and NG > 1)
                    step = NS if fine else GROW
                    nchunk = GROW // step
                    xrf = xr.rearrange('p b s -> p (b s)')
                    ost = work.tile([128, 2 * NG, HW], F32, tag='outst',
                                    name=f'ost_{mp}_{g}', bufs=3)
                    osf = ost.rearrange('p b s -> p (b s)')
                    for c in range(nchunk):
                        sl = slice(c * step, (c + 1) * step)
                        nb = step // NS
                        v_row = w2pool[0].tile(
                            [128, step], F32, tag='rowF',
                            name=f'v3_{mp}_{g}_{c}', bufs=4,
                            padded_shape=[128, GROW])
                        for i in range(nb):
                            nc.scalar.activation(
                                v_row[:, i * NS:(i + 1) * NS],
                                psb[c * nb + i], AF.Identity,
                                bias=B3[:, mp:mp + 1],
                                scale=A3[:, mp:mp + 1])
                        u_row = w2pool[0].tile(
                            [128, step], F32, tag='rowE',
                            name=f'u3_{mp}_{g}_{c}', bufs=4,
                            padded_shape=[128, GROW])
                        nc.vector.scalar_tensor_tensor(
                            u_row, xrf[:, sl], k3c, v_row,
                            op0=ALU.mult, op1=ALU.add)
                        d3 = w2pool[0].tile(
                            [128, step], F32, tag='rowC',
                            name=f'd3_{mp}_{g}_{c}', bufs=4,
                            padded_shape=[128, GROW])
                        d3e = nc.gpsimd if mp % 2 == 0 else nc.vector
                        d3e.tensor_scalar(d3, u_row, MAGIC,
                                          MAGIC + 255.0,
                                          op0=ALU.add, op1=ALU.min)
                        r3 = w2pool[0].tile(
                            [128, step], F32, tag='rowD',
                            name=f'r3_{mp}_{g}_{c}', bufs=4,
                            padded_shape=[128, GROW])
                        nc.vector.tensor_scalar(r3, d3, MAGIC, MAGIC,
                                                op0=ALU.max,
                                                op1=ALU.subtract)
                        if mp % 2 == 0:
                            nc.scalar.mul(osf[:, sl], r3, S3[:, mp:mp + 1])
                        else:
                            nc.vector.tensor_scalar(osf[:, sl], r3,
                                                    S3[:, mp:mp + 1], None,
                                                    op0=ALU.mult)
                        nc.sync.dma_start(
                            out_d[mp * 128:(mp + 1) * 128,
                                  2 * NG * g + 2 * c * nb:
                                  2 * NG * g + 2 * (c + 1) * nb, :],
                            ost[:, 2 * c * nb:2 * (c + 1) * nb, :])


            # ---- emission schedule: group pipeline ----
            w2pool = []
            for p in range(MP1):
                nc.gpsimd.memset(pad1[p], 0.0)
            for g_ in range(GRP):
                conv1g(g_)
            mark('tr2')
            I2T = {}
            for mp in range(MP1):
                blk = I2p[mp].rearrange('p (c t) -> p t c', t=9)
                pairs = [(tap, ci) for tap in range(9) for ci in range(MP1)]
                for c0 in range(0, len(pairs), 4):
                    chunk = pairs[c0:c0 + 4]
                    pt = pst.tile([128, 128 * len(chunk)], BF16, tag='pst',
                                  name=f'ptr2_{mp}_{c0}',
                                  padded_shape=[128, 512])
                    big = data.tile([128, 128 * len(chunk)], BF16,
                                    name=f'I2Tb_{mp}_{c0}')
                    for j, (tap, ci) in enumerate(chunk):
                        nc.tensor.transpose(
                            pt[:, j * 128:(j + 1) * 128],
                            blk[:, tap, ci * 128:(ci + 1) * 128], ident)
                        I2T[(tap, ci, mp)] = big[:, j * 128:(j + 1) * 128]
                    nc.scalar.copy(big, pt)

            I3T = [data.tile([128, COUT], BF16, name=f'I3T_{ci}')
                   for ci in range(MP1)]
            for ci in range(MP1):
                pt = pst.tile([128, COUT], BF16, tag='pst', name=f'ptr3_{ci}')
                for mp in range(MP3):
                    nc.tensor.transpose(
                        pt[:, mp * 128:(mp + 1) * 128],
                        I3p[mp][:, ci * 128:(ci + 1) * 128], ident)
                nc.scalar.copy(I3T[ci], pt)

            xstack.close()  # release xsplit; work2 reuses its space
            w2pool.append(tc.alloc_tile_pool(name='work2', bufs=4))
            mark('conv2')
            conv2g(0)
            mark('conv3')
            conv3g(0, 0, 6)
            conv2g(1)
            conv3g(0, 6, MP3)
            conv3g(1)
            w2pool[0].release()

    mark('end')
    nc.finalize()
    return nc


_NC_CACHE = {}


def _get_nc(a1c, a2c, a3c):
    key = (a1c, a2c, a3c)
    if key not in _NC_CACHE:
        _NC_CACHE[key] = build_nc(a1c, a2c, a3c)
    return _NC_CACHE[key]


def run_all(inputs, trace=False, **kw):
    # host relayout (pure transpose) + bf16 hi/lo re-encoding of x
    x = np.asarray(inputs['x'], np.float32).reshape(8, B, CIN, HW)
    x = np.ascontiguousarray(x.transpose(0, 2, 1, 3))  # [core, CIN, B, HW]
    xh = x.astype(ml_dtypes.bfloat16)
    xl = (x - xh.astype(np.float32)).astype(ml_dtypes.bfloat16)
    xhl = np.ascontiguousarray(np.stack([xh, xl], axis=2))  # [core,CIN,2,B,HW]

    w1 = np.ascontiguousarray(inputs['w1'].reshape(WID, CIN), np.float32)
    w2 = np.ascontiguousarray(inputs['w2'].reshape(WID, WID * 9), np.float32)
    w3 = np.ascontiguousarray(inputs['w3'].reshape(COUT, WID), np.float32)
    a1 = np.asarray(inputs['a1'])
    a2 = np.asarray(inputs['a2'])
    assert np.all(a1 == a1[0]), "kernel assumes constant a1 (PACT alpha)"
    assert np.all(a2 == a2[0]), "kernel assumes constant a2 (PACT alpha)"
    a3 = np.asarray(inputs['a3'])
    assert np.all(a3 == a3[0]), "kernel assumes constant a3 (PACT alpha)"
    nc = _get_nc(float(a1[0]), float(a2[0]), float(a3[0]))

    base = dict(w1=w1, w2=w2, w3=w3)
    for l in ('1', '2', '3'):
        cols = []
        for nm in ('g', 'b', 'm', 'v', 'a'):
            p = np.asarray(inputs[nm + l], np.float32)
            cols.append(p.reshape(-1, 128).T)  # [128, P]
        base['p' + l] = np.ascontiguousarray(np.concatenate(cols, axis=1))
    in_maps = [dict(base, x=x[c], xhl=xhl[c]) for c in range(8)]
    res = run_bass_kernel_spmd(nc, in_maps, core_ids=list(range(8)),
                               trace=trace, **kw)
    out = np.stack([r['out'].transpose(1, 0, 2) for r in res.results])
    return out.reshape(64, COUT, 14, 14), res


def kernel(**inputs):
    out, _ = run_all(inputs)
    return out

